# revision 31
# baseline (speedup 1.0000x reference)
"""Trainium2 Bass kernel: inclusive cumsum along L for X (4, 8192, 32, 32) f32.

Strategy (8 NeuronCores, SPMD) — matmul-scan, no transposes, no DVE scan:
  - View X as (B=4, L=8192, C=1024) with C = D*N flattened. Shard the 4096
    independent scan rows (b, c) 8 ways: core i gets b = i//2 and c-half
    h = i%2, i.e. a (8192, 512) f32 slab with 2 KiB-contiguous DRAM rows.
  - Per core, 16 superblocks of 512 L-rows in NATURAL (l, c) layout
    (partition = l within each 128-row block). Per superblock (12 matmuls,
    all rhs free = 512, lhsT constants loaded once from a host-supplied
    "w" input so gpsimd needs no ucode):
      * UT^T @ xb_ks           within-128-block inclusive cumsum (4 mms)
      * ALLONES^T @ xprefix    intra-superblock carries; xprefix = bf16
                               running sums xb0+..+xb_j built by 3 DVE adds
                               (3 mms + 1 colsum mm feeding the chain)
      * ones_row^T @ hi        inter-superblock carry broadcast (4 mms),
                               hi = bf16(S); S itself stays fp32 in a tiny
                               serial DVE chain S += colsum (no compounding
                               rounding), so worst-case |err| ~1.5 vs the
                               ~9 abs budget (2e-2 of output scale ~456).
  - ACT: f32->bf16 conversion + 3 PSUM evacuations; DVE: prefix adds,
    carry chain, 1 evacuation. PE/ACT/DVE all ~47-67us, under the ~92us
    HBM floor (268 MB over 8 cores at ~2.9 TB/s device bandwidth).
  - DMA: in-DMAs on sync (HWDGE queue), out-DMAs on gpsimd (SWDGE), 1 MiB
    each; read-ahead capped by xt bufs so read and write streams stay
    overlapped end-to-end (~390 GB/s/core effective vs ~262 write-only).
  - Measured: ~86-100us DMA-active per core + ~8.6us NEFF/prologue
    preamble + ~9us Tile exit barrier -> ~105-118us/core, mean ~103-108,
    graded max-core 112-118 (HBM-arbitration luck dominates the spread).
"""

import numpy as np
import ml_dtypes
from contextlib import ExitStack

import concourse.bass as bass
import concourse.tile as tile
from concourse import bacc, mybir
from concourse.bass_utils import run_bass_kernel_spmd

N_CORES = 8
B, L, D, N = 4, 8192, 32, 32
C_FULL = D * N          # 1024 columns per batch entry
C = C_FULL // 2         # 512 columns per core
P = 128                 # partitions
SUPER = 512             # L rows per superblock
N_SUPER = L // SUPER    # 16
BLKS = SUPER // P       # 4 blocks per superblock

_CACHE = {}


def _build_program():
    f32 = mybir.dt.float32
    bf16 = mybir.dt.bfloat16
    add = mybir.AluOpType.add
    sub = mybir.AluOpType.subtract
    nc = bacc.Bacc(
        trn_type="TRN2", debug=False, num_devices=N_CORES, num_swdge_queues=1
    )
    x = nc.dram_tensor("x", [L, C], f32, kind="ExternalInput").ap()
    # w[:, 0:128] = upper-triangular ones (incl diag), w[:, 128:256] = ones.
    # Loaded from the host so gpsimd stays instruction-free (cheap exit drain).
    w = nc.dram_tensor("w", [P, 2 * P], bf16, kind="ExternalInput").ap()
    y = nc.dram_tensor("y", [L, C], f32, kind="ExternalOutput").ap()

    with tile.TileContext(nc) as tc, ExitStack() as ctx:
        const_pool = ctx.enter_context(tc.tile_pool(name="const", bufs=1))
        xt_pool = ctx.enter_context(tc.tile_pool(name="xt", bufs=3))
        xb_pool = ctx.enter_context(tc.tile_pool(name="xb", bufs=3))
        xp_pool = ctx.enter_context(tc.tile_pool(name="xp", bufs=2))
        yt_pool = ctx.enter_context(tc.tile_pool(name="yt", bufs=3))
        s_pool = ctx.enter_context(tc.tile_pool(name="s", bufs=1))
        ps_pool = ctx.enter_context(tc.tile_pool(name="ps", bufs=1, space="PSUM"))
        pss_pool = ctx.enter_context(tc.tile_pool(name="pss", bufs=1, space="PSUM"))

        # UT[k, m] = 1 iff k <= m  ->  (UT^T @ x)[m] = sum_{k<=m} x[k]
        wsb = const_pool.tile([P, 2 * P], bf16, name="wsb")
        nc.gpsimd.dma_start(out=wsb[:], in_=w[:, :])
        ut = wsb[:, 0:P]
        ao = wsb[:, P : 2 * P]

        s_cur = s_pool.tile([1, C], f32, name="s0")  # fp32 carry into superblock t
        nc.vector.memset(s_cur[:], 0.0)
        for t in range(N_SUPER):
            # Last two superblocks run at 128-row quarter granularity so only
            # the final 256 KiB quarter's conv->mm->evac->DMA chain (~3us)
            # trails the last read byte, instead of the whole superblock's.
            fine = t >= N_SUPER - 2 or t == 0

            # ---- load superblock: one 1 MiB DMA, element order [p][ks][c]
            xt = xt_pool.tile([P, BLKS * C], f32, name="xt", tag="xt", bufs=3)
            if fine:
                for ks in range(BLKS):
                    nc.sync.dma_start(
                        out=xt[:, ks * C : (ks + 1) * C],
                        in_=x[t * SUPER + ks * P : t * SUPER + (ks + 1) * P, :],
                    )
            else:
                src = x[t * SUPER : (t + 1) * SUPER, :].rearrange(
                    "(ks p) c -> p ks c", p=P
                )
                nc.sync.dma_start(
                    out=xt[:].rearrange("p (ks c) -> p ks c", ks=BLKS), in_=src
                )

            # ---- f32 -> bf16
            xb = xb_pool.tile([P, BLKS * C], bf16, name="xb", tag="xb", bufs=3)
            if fine:
                for ks in range(BLKS):
                    nc.scalar.copy(
                        xb[:, ks * C : (ks + 1) * C], xt[:, ks * C : (ks + 1) * C]
                    )
            else:
                nc.scalar.copy(xb[:], xt[:])

            # ---- bf16 prefix tiles: xp[j] = xb_0 + .. + xb_j (DVE)
            # lets one ALLONES matmul apply the full intra-superblock carry
            xp = xp_pool.tile([P, (BLKS - 1) * C], bf16, name="xp", tag="xp", bufs=2)
            nc.vector.tensor_tensor(
                xp[:, 0:C], xb[:, 0:C], xb[:, C : 2 * C], add
            )
            for j in range(1, BLKS - 1):
                nc.vector.tensor_tensor(
                    xp[:, j * C : (j + 1) * C],
                    xp[:, (j - 1) * C : j * C],
                    xb[:, (j + 1) * C : (j + 2) * C],
                    add,
                )

            # ---- PE: per-block cumsum + carries, grouped by stationary tensor
            # 4 one-bank PSUM tiles; block 3 single-buffered (8 banks total
            # with pss): q0-q2 x2 + q3 x1 + pss x1.
            ph = [
                ps_pool.tile(
                    [P, C], f32, name=f"q{ks}", tag=f"q{ks}",
                    bufs=(2 if ks < 3 else 1),
                )
                for ks in range(BLKS)
            ]

            def region(ks):
                return ph[ks][:]

            def nmm(ks):  # matmuls accumulating into region ks
                return 1 + (1 if ks > 0 else 0) + (1 if t > 0 else 0)

            done = [0] * BLKS

            def flags(ks):
                done[ks] += 1
                return dict(
                    start=(done[ks] == 1),
                    stop=(done[ks] == nmm(ks)),
                    skip_group_check=True,
                )

            if t < N_SUPER - 1:
                # superblock column sum -> [1, C]; feeds the carry chain
                pss = pss_pool.tile([1, C], f32, name="pss", tag="pss", bufs=1)
                nc.tensor.matmul(
                    pss[:], lhsT=wsb[:, P : P + 1], rhs=xp[:, 2 * C : 3 * C],
                    start=True, stop=True, skip_group_check=True,
                )
            for ks in range(BLKS):  # within-block cumsum
                nc.tensor.matmul(
                    region(ks), lhsT=ut, rhs=xb[:, ks * C : (ks + 1) * C],
                    **flags(ks),
                )
            # intra-superblock carries: block ks gets colsum(xb_0+..+xb_{ks-1})
            nc.tensor.matmul(region(1), lhsT=ao, rhs=xb[:, 0:C], **flags(1))
            for ks in (2, 3):
                nc.tensor.matmul(
                    region(ks), lhsT=ao, rhs=xp[:, (ks - 2) * C : (ks - 1) * C],
                    **flags(ks),
                )
            if t > 0:  # inter-superblock carry (bf16 round of fp32 S)
                for ks in range(BLKS):
                    nc.tensor.matmul(
                        region(ks), lhsT=wsb[0:1, P : 2 * P], rhs=hi[:], **flags(ks)
                    )

            # ---- next carry: S' = S + column sums (fp32, serial chain)
            if t < N_SUPER - 1:
                s_next = s_pool.tile([1, C], f32, name="s", tag="s", bufs=2)
                nc.vector.tensor_tensor(s_next[:], s_cur[:], pss[:], add)
                s_cur = s_next
                hi = s_pool.tile([1, C], bf16, name="hi", tag="hi", bufs=2)
                nc.vector.tensor_copy(hi[:], s_cur[:])

            # ---- evacuate PSUM -> SBUF (q0-q2 on ACT, q3 on DVE), 1 MiB out
            yt = yt_pool.tile([P, BLKS * C], f32, name="yt", tag="yt", bufs=3)
            for ks in range(BLKS - 1):
                nc.scalar.copy(yt[:, ks * C : (ks + 1) * C], ph[ks][:])
            nc.vector.tensor_copy(yt[:, 3 * C : 4 * C], ph[3][:])
            if fine:
                for ks in range(BLKS):
                    nc.gpsimd.dma_start(
                        out=y[t * SUPER + ks * P : t * SUPER + (ks + 1) * P, :],
                        in_=yt[:, ks * C : (ks + 1) * C],
                    )
            else:
                ydst = y[t * SUPER : (t + 1) * SUPER, :].rearrange(
                    "(ks p) c -> p ks c", p=P
                )
                nc.gpsimd.dma_start(
                    out=ydst, in_=yt[:].rearrange("p (ks c) -> p ks c", ks=BLKS)
                )

    nc.compile()
    return nc


def _get_program():
    if "nc" not in _CACHE:
        _CACHE["nc"] = _build_program()
    return _CACHE["nc"]


def _shard(X):
    """(4, 8192, 32, 32) -> 8 contiguous (8192, 512) slabs."""
    Xv = X.reshape(B, L, C_FULL)
    shards = []
    for i in range(N_CORES):
        b, h = i // 2, i % 2
        shards.append(np.ascontiguousarray(Xv[b, :, h * C : (h + 1) * C]))
    return shards


def _unshard(parts):
    out = np.empty((B, L, C_FULL), dtype=np.float32)
    for i in range(N_CORES):
        b, h = i // 2, i % 2
        out[b, :, h * C : (h + 1) * C] = parts[i]
    return out.reshape(B, L, D, N)


def _make_w():
    w = np.zeros((P, 2 * P), dtype=ml_dtypes.bfloat16)
    w[:, 0:P] = np.triu(np.ones((P, P), dtype=np.float32)).astype(ml_dtypes.bfloat16)
    w[:, P : 2 * P] = 1
    return w


def kernel(X_in, _trace=False, _tmpdir=None, _trace_cores=None):
    X = np.asarray(X_in, dtype=np.float32)
    assert X.shape == (B, L, D, N), X.shape
    nc = _get_program()
    w = _make_w()
    in_maps = [{"x": s, "w": w} for s in _shard(X)]
    kwargs = {}
    if _trace:
        kwargs = dict(
            trace=True,
            tmpdir=_tmpdir,
            trace_cores=_trace_cores or list(range(N_CORES)),
        )
    res = run_bass_kernel_spmd(nc, in_maps, core_ids=list(range(N_CORES)), **kwargs)
    out = _unshard([res.results[i]["y"] for i in range(N_CORES)])
    kernel.last_results = res
    return out
